# revision 1
# baseline (speedup 1.0000x reference)
"""Trainium2 Bass kernel for nn_DenoisedSasrec (GAU-style sparse attention).

Contract: kernel(**inputs) takes FULL unsharded numpy inputs (as produced by
setup_inputs) and returns the FULL [64, 512, 512] float32 output.

Strategy (data-parallel over batch, per sharding hint):
  - 64 batch items are sharded 8-per-core across the 8 NeuronCores.
  - The item-embedding table, projection weights and the [L,L] sparse-mask
    constants are replicated to every core.
  - Per batch item, on device:
      X  = gather(item_emb, positives)                 (indirect DMA)
      XT = X^T (+ pos_emb^T fused into PSUM evacuation) (PE transposes)
      Z^T = silu(Wz @ X^T), V = X @ Wv^T (silu)        (PE + ACT)
      Q^T = (Wq @ Z^T)*gamma_q+beta_q, K^T likewise    (PE + ACT)
      P^T = K^T^T-contracted attention logits          (PE)
      A^T = (relu(P^T) * S_b)^2                        (DVE, one fused pass + square)
      OUT = A @ V                                      (PE)
  where S_b[j,l] = smask[l,j]*keep_b[l,j]/sqrt(L*H) is built per batch from
  two host constants and the per-key padding mask (keep = diag OR
  (mask[j] AND j<=l)); the mask/smask/relu^2/(L*H) algebra of the reference
  folds exactly into (relu(P) * S)^2 because smask>0 and keep is 0/1.
"""

import numpy as np

import concourse.bass as bass
import concourse.mybir as mybir
import concourse.tile as tile
from concourse import bacc
from concourse.bass_utils import run_bass_kernel_spmd
from concourse.masks import make_identity

B, L, H = 64, 512, 512
ITEM = 50001
TEMP = 0.2
N_CORES = 8
BPC = B // N_CORES  # batches per core
P = 128
NC_CHUNKS = L // P  # 4

F32 = mybir.dt.float32
I32 = mybir.dt.int32

_COMPILED = None  # cache (nc) across calls


def _build_module():
    nc = bacc.Bacc("TRN2", target_bir_lowering=False, debug=False)

    # ---- DRAM I/O ----
    d_pos = nc.dram_tensor("positives", [BPC, L], I32, kind="ExternalInput")
    d_msk = nc.dram_tensor("maskf", [BPC, L], F32, kind="ExternalInput")
    d_emb = nc.dram_tensor("item_emb", [ITEM, H], F32, kind="ExternalInput")
    d_post = nc.dram_tensor("PosT", [H, L], F32, kind="ExternalInput")
    d_wzt = nc.dram_tensor("WzT", [H, H], F32, kind="ExternalInput")
    d_wvt = nc.dram_tensor("WvT", [H, H], F32, kind="ExternalInput")
    d_wqt = nc.dram_tensor("WqT", [H, H], F32, kind="ExternalInput")
    d_wkt = nc.dram_tensor("WkT", [H, H], F32, kind="ExternalInput")
    d_m1s = nc.dram_tensor("M1s", [L, L], F32, kind="ExternalInput")
    d_ds = nc.dram_tensor("Ds", [L, L], F32, kind="ExternalInput")
    d_gq = nc.dram_tensor("gq", [H], F32, kind="ExternalInput")
    d_bq = nc.dram_tensor("bq", [H], F32, kind="ExternalInput")
    d_gk = nc.dram_tensor("gk", [H], F32, kind="ExternalInput")
    d_bk = nc.dram_tensor("bk", [H], F32, kind="ExternalInput")
    d_out = nc.dram_tensor("out", [BPC, L, H], F32, kind="ExternalOutput")

    AF = mybir.ActivationFunctionType
    OP = mybir.AluOpType

    with tile.TileContext(nc) as tc:
        with (
            tc.tile_pool(name="const", bufs=1) as cpool,
            tc.tile_pool(name="io", bufs=2) as iopool,
            tc.tile_pool(name="acts", bufs=2) as apool,
            tc.tile_pool(name="small", bufs=3) as smpool,
            tc.tile_pool(name="psum", bufs=4, space="PSUM") as pspool,
            tc.tile_pool(name="psumt", bufs=2, space="PSUM") as tppool,
        ):
            # ---- constants into SBUF ----
            ident = cpool.tile([P, P], F32, name="ident")
            make_identity(nc, ident[:])

            def load_chunks(dram, name):
                ts = []
                for c in range(NC_CHUNKS):
                    t = cpool.tile([P, L], F32, name=f"{name}{c}", tag=f"{name}{c}")
                    nc.sync.dma_start(out=t[:], in_=dram[c * P:(c + 1) * P, :])
                    ts.append(t)
                return ts

            WzT = load_chunks(d_wzt, "wz")
            WvT = load_chunks(d_wvt, "wv")
            WqT = load_chunks(d_wqt, "wq")
            WkT = load_chunks(d_wkt, "wk")
            PosT = load_chunks(d_post, "pt")
            M1s = load_chunks(d_m1s, "m1")
            Ds = load_chunks(d_ds, "ds")

            def load_vec(dram, name):
                t = cpool.tile([P, NC_CHUNKS], F32, name=name)
                nc.sync.dma_start(
                    out=t[:], in_=dram.ap().rearrange("(c p) -> p c", p=P)
                )
                return t

            gq = load_vec(d_gq, "gq")
            bq = load_vec(d_bq, "bq")
            gk = load_vec(d_gk, "gk")
            bk = load_vec(d_bk, "bk")

            emb_ap = d_emb.ap()

            for b in range(BPC):
                # per-batch index / mask vectors: [128, 4] (p fastest in dram)
                idx = iopool.tile([P, NC_CHUNKS], I32, name=f"idx{b}", tag="idx")
                nc.sync.dma_start(
                    out=idx[:], in_=d_pos.ap()[b].rearrange("(c p) -> p c", p=P)
                )
                msk = iopool.tile([P, NC_CHUNKS], F32, name=f"msk{b}", tag="msk")
                nc.sync.dma_start(
                    out=msk[:], in_=d_msk.ap()[b].rearrange("(c p) -> p c", p=P)
                )

                # ---- gather X = item_emb[positives[b]] : 4x [128(l), 512(k)] ----
                X = []
                for lc in range(NC_CHUNKS):
                    xt_ = iopool.tile([P, H], F32, name=f"x{b}_{lc}", tag=f"x{lc}")
                    nc.gpsimd.indirect_dma_start(
                        out=xt_[:],
                        out_offset=None,
                        in_=emb_ap,
                        in_offset=bass.IndirectOffsetOnAxis(
                            ap=idx[:, lc:lc + 1], axis=0
                        ),
                    )
                    X.append(xt_)

                # ---- XT = X^T + PosT : 4x [128(k), 512(l)] ----
                XT = []
                for kc in range(NC_CHUNKS):
                    tp = tppool.tile([P, L], F32, name=f"tp{b}_{kc}", tag="tp")
                    for lc in range(NC_CHUNKS):
                        nc.tensor.transpose(
                            out=tp[:, lc * P:(lc + 1) * P],
                            in_=X[lc][:, kc * P:(kc + 1) * P],
                            identity=ident[:],
                        )
                    xtt = apool.tile([P, L], F32, name=f"xt{b}_{kc}", tag=f"xt{kc}")
                    nc.vector.tensor_add(out=xtt[:], in0=tp[:], in1=PosT[kc][:])
                    XT.append(xtt)

                # ---- Z^T[h,l] = silu(sum_k Wz[h,k] XT[k,l]) ----
                ZT = []
                for hc in range(NC_CHUNKS):
                    zp = pspool.tile([P, L], F32, name=f"zp{b}_{hc}", tag="mm")
                    for kc in range(NC_CHUNKS):
                        nc.tensor.matmul(
                            out=zp[:],
                            lhsT=WzT[kc][:, hc * P:(hc + 1) * P],
                            rhs=XT[kc][:],
                            start=(kc == 0),
                            stop=(kc == NC_CHUNKS - 1),
                        )
                    zt = apool.tile([P, L], F32, name=f"zt{b}_{hc}", tag=f"zt{hc}")
                    nc.scalar.activation(out=zt[:], in_=zp[:], func=AF.Silu)
                    ZT.append(zt)

                # ---- V[l,h] = silu(sum_k XT[k,l] WvT[k,h]) ----
                V = []
                for lc in range(NC_CHUNKS):
                    vp = pspool.tile([P, L], F32, name=f"vp{b}_{lc}", tag="mm")
                    for kc in range(NC_CHUNKS):
                        nc.tensor.matmul(
                            out=vp[:],
                            lhsT=XT[kc][:, lc * P:(lc + 1) * P],
                            rhs=WvT[kc][:],
                            start=(kc == 0),
                            stop=(kc == NC_CHUNKS - 1),
                        )
                    vt = apool.tile([P, L], F32, name=f"v{b}_{lc}", tag=f"v{lc}")
                    nc.scalar.activation(out=vt[:], in_=vp[:], func=AF.Silu)
                    V.append(vt)

                # ---- Q^T = (Wq @ Z^T) * gamma_q + beta_q ; K^T likewise ----
                QT, KT = [], []
                for (wt, gam, bet, outl, nm) in (
                    (WqT, gq, bq, QT, "q"),
                    (WkT, gk, bk, KT, "k"),
                ):
                    for hc in range(NC_CHUNKS):
                        qp = pspool.tile([P, L], F32, name=f"{nm}p{b}_{hc}", tag="mm")
                        for kc in range(NC_CHUNKS):
                            nc.tensor.matmul(
                                out=qp[:],
                                lhsT=wt[kc][:, hc * P:(hc + 1) * P],
                                rhs=ZT[kc][:],
                                start=(kc == 0),
                                stop=(kc == NC_CHUNKS - 1),
                            )
                        qt = apool.tile(
                            [P, L], F32, name=f"{nm}t{b}_{hc}", tag=f"{nm}t{hc}"
                        )
                        nc.scalar.activation(
                            out=qt[:],
                            in_=qp[:],
                            func=AF.Identity,
                            scale=gam[:, hc:hc + 1],
                            bias=bet[:, hc:hc + 1],
                        )
                        outl.append(qt)

                # ---- S_b[j,l] = M1s[j,l]*mask[j] + Ds[j,l] ----
                S = []
                for mc in range(NC_CHUNKS):
                    st = apool.tile([P, L], F32, name=f"s{b}_{mc}", tag=f"s{mc}")
                    nc.vector.scalar_tensor_tensor(
                        out=st[:],
                        in0=M1s[mc][:],
                        scalar=msk[:, mc:mc + 1],
                        in1=Ds[mc][:],
                        op0=OP.mult,
                        op1=OP.add,
                    )
                    S.append(st)

                # ---- P^T[m,l] = sum_d KT[d,m] QT[d,l] ; A^T = (relu*S)^2 ----
                A = []
                for mc in range(NC_CHUNKS):
                    pp = pspool.tile([P, L], F32, name=f"pp{b}_{mc}", tag="mm")
                    for dc in range(NC_CHUNKS):
                        nc.tensor.matmul(
                            out=pp[:],
                            lhsT=KT[dc][:, mc * P:(mc + 1) * P],
                            rhs=QT[dc][:],
                            start=(dc == 0),
                            stop=(dc == NC_CHUNKS - 1),
                        )
                    u = smpool.tile([P, L], F32, name=f"u{b}_{mc}", tag="u")
                    nc.vector.scalar_tensor_tensor(
                        out=u[:],
                        in0=pp[:],
                        scalar=0.0,
                        in1=S[mc][:],
                        op0=OP.max,
                        op1=OP.mult,
                    )
                    at = apool.tile([P, L], F32, name=f"a{b}_{mc}", tag=f"a{mc}")
                    nc.vector.tensor_mul(out=at[:], in0=u[:], in1=u[:])
                    A.append(at)

                # ---- OUT[l,h] = sum_m A[m,l] V[m,h] ----
                for lc in range(NC_CHUNKS):
                    op_ = pspool.tile([P, L], F32, name=f"op{b}_{lc}", tag="mm")
                    for mc in range(NC_CHUNKS):
                        nc.tensor.matmul(
                            out=op_[:],
                            lhsT=A[mc][:, lc * P:(lc + 1) * P],
                            rhs=V[mc][:],
                            start=(mc == 0),
                            stop=(mc == NC_CHUNKS - 1),
                        )
                    ot = smpool.tile([P, L], F32, name=f"o{b}_{lc}", tag="o")
                    nc.scalar.copy(out=ot[:], in_=op_[:])
                    nc.sync.dma_start(
                        out=d_out.ap()[b, lc * P:(lc + 1) * P, :], in_=ot[:]
                    )

    nc.compile()
    return nc


def _host_prep(positives, mask, item_emb, pos_emb, Wz, Wv, Wq, Wk,
               gamma_q, beta_q, gamma_k, beta_k, sparse_w, gumbel):
    """Small O(L^2) constant prep + per-core input shards."""
    f32 = np.float32
    positives = np.ascontiguousarray(np.asarray(positives).astype(np.int32))
    maskf = np.ascontiguousarray(np.asarray(mask).astype(f32))
    item_emb = np.ascontiguousarray(np.asarray(item_emb, f32))
    pos_emb = np.asarray(pos_emb, f32)
    sw = np.asarray(sparse_w, f32)
    gum = np.asarray(gumbel, f32)

    smask = (1.0 / (1.0 + np.exp(-((np.log(sw / (1.0 - sw)) + gum) / f32(TEMP)))))
    smask = smask.astype(f32)
    scl = f32(1.0 / np.sqrt(L * H))
    j = np.arange(L)
    strict_lower_T = (j[:, None] < j[None, :])  # [j, l] : j < l
    M1s = np.ascontiguousarray((smask.T * strict_lower_T * scl).astype(f32))
    Ds = np.ascontiguousarray((np.diag(np.diag(smask)) * scl).astype(f32))

    shared = {
        "item_emb": item_emb,
        "PosT": np.ascontiguousarray(pos_emb.T.astype(f32)),
        "WzT": np.ascontiguousarray(np.asarray(Wz, f32).T),
        "WvT": np.ascontiguousarray(np.asarray(Wv, f32).T),
        "WqT": np.ascontiguousarray(np.asarray(Wq, f32).T),
        "WkT": np.ascontiguousarray(np.asarray(Wk, f32).T),
        "M1s": M1s,
        "Ds": Ds,
        "gq": np.ascontiguousarray(np.asarray(gamma_q, f32)),
        "bq": np.ascontiguousarray(np.asarray(beta_q, f32)),
        "gk": np.ascontiguousarray(np.asarray(gamma_k, f32)),
        "bk": np.ascontiguousarray(np.asarray(beta_k, f32)),
    }
    in_maps = []
    for c in range(N_CORES):
        sl = slice(c * BPC, (c + 1) * BPC)
        m = dict(shared)
        m["positives"] = positives[sl]
        m["maskf"] = maskf[sl]
        in_maps.append(m)
    return in_maps


def get_module():
    global _COMPILED
    if _COMPILED is None:
        _COMPILED = _build_module()
    return _COMPILED


def kernel(**inputs) -> np.ndarray:
    nc = get_module()
    in_maps = _host_prep(**inputs)
    res = run_bass_kernel_spmd(nc, in_maps, core_ids=list(range(N_CORES)))
    out = np.concatenate([r["out"] for r in res.results], axis=0)
    return out.astype(np.float32)


if __name__ == "__main__":
    rng = np.random.default_rng(0)
    demo = {
        "positives": rng.integers(0, ITEM, (B, L)).astype(np.int32),
        "mask": rng.integers(0, 2, (B, L)).astype(np.int32),
        "item_emb": rng.normal(size=(ITEM, H)).astype(np.float32) * 0.02,
        "pos_emb": rng.normal(size=(L, H)).astype(np.float32) * 0.02,
        "Wz": rng.normal(size=(L, L)).astype(np.float32),
        "Wv": rng.normal(size=(L, L)).astype(np.float32),
        "Wq": rng.normal(size=(L, L)).astype(np.float32),
        "Wk": rng.normal(size=(L, L)).astype(np.float32),
        "gamma_q": rng.normal(size=(L,)).astype(np.float32) * 0.02,
        "beta_q": np.zeros((L,), np.float32),
        "gamma_k": rng.normal(size=(L,)).astype(np.float32) * 0.02,
        "beta_k": np.zeros((L,), np.float32),
        "sparse_w": rng.uniform(0.2, 0.8, (L, H)).astype(np.float32),
        "gumbel": rng.normal(size=(L, H)).astype(np.float32),
    }
    out = kernel(**demo)
    print("out", out.shape, out.dtype, np.abs(out).max())


# revision 10
# speedup vs baseline: 3.7049x; 3.7049x over previous
"""Trainium2 Bass kernel for nn_DenoisedSasrec (GAU-style sparse attention).

Contract: kernel(**inputs) takes FULL unsharded numpy inputs (as produced by
setup_inputs) and returns the FULL [64, 512, 512] float32 output.

Strategy (data-parallel over batch, per sharding hint):
  - 64 batch items are sharded 8-per-core across the 8 NeuronCores.
  - The item-embedding table, projection weights and the [L,L] sparse-mask
    constants are replicated to every core.
  - Per batch item, on device:
      X  = gather(item_emb, positives)                 (indirect DMA)
      XT = X^T (+ pos_emb^T fused into PSUM evacuation) (PE transposes)
      Z^T = silu(Wz @ X^T), V = X @ Wv^T (silu)        (PE + ACT)
      Q^T = (Wq @ Z^T)*gamma_q+beta_q, K^T likewise    (PE + ACT)
      P^T = K^T^T-contracted attention logits          (PE)
      A^T = (relu(P^T) * S_b)^2                        (DVE, one fused pass + square)
      OUT = A @ V                                      (PE)
  where S_b[j,l] = smask[l,j]*keep_b[l,j]/sqrt(L*H) is built per batch from
  two host constants and the per-key padding mask (keep = diag OR
  (mask[j] AND j<=l)); the mask/smask/relu^2/(L*H) algebra of the reference
  folds exactly into (relu(P) * S)^2 because smask>0 and keep is 0/1.

  Performance notes:
  - All GEMMs run in dt.float32r (full-rate PE streaming, 4x over fp32;
    measured end-to-end rel err ~4e-4 vs the fp32 reference).
  - Attention is causal: for key-chunk mc, columns l < 128*mc of A^T are
    exactly zero (S=0 there), so the P/A/S work shrinks to the live range
    and 6 of 16 OUT matmuls per item are skipped — exact, no approximation.
  - The per-item stages are software-pipelined: item b+1's gather and PE
    transposes are emitted inside item b's attention phase.
  - Modeled (TimelineSim) per-core time: ~191 us, PE ~86% busy.
"""

import numpy as np

import concourse.bass as bass
import concourse.mybir as mybir
import concourse.tile as tile
from concourse import bacc
from concourse.bass_utils import run_bass_kernel_spmd

B, L, H = 64, 512, 512
ITEM = 50001
TEMP = 0.2
N_CORES = 8
BPC = B // N_CORES  # batches per core
P = 128
NC_CHUNKS = L // P  # 4

F32 = mybir.dt.float32
F32R = mybir.dt.float32r
I32 = mybir.dt.int32


_COMPILED = None  # cache (nc) across calls


def _build_module():
    nc = bacc.Bacc("TRN2", target_bir_lowering=False, debug=False)

    # ---- DRAM I/O ----
    d_pos = nc.dram_tensor("positives", [BPC, L], I32, kind="ExternalInput")
    d_msk = nc.dram_tensor("maskf", [BPC, L], F32, kind="ExternalInput")
    d_emb = nc.dram_tensor("item_emb", [ITEM, H], F32R, kind="ExternalInput")
    d_post = nc.dram_tensor("PosT", [H, L], F32, kind="ExternalInput")
    d_wzt = nc.dram_tensor("WzT", [H, H], F32R, kind="ExternalInput")
    d_wvt = nc.dram_tensor("WvT", [H, H], F32R, kind="ExternalInput")
    d_wqt = nc.dram_tensor("WqT", [H, H], F32R, kind="ExternalInput")
    d_wkt = nc.dram_tensor("WkT", [H, H], F32R, kind="ExternalInput")
    d_m1s = nc.dram_tensor("M1s", [L, L], F32, kind="ExternalInput")
    d_ds = nc.dram_tensor("Ds", [L, L], F32, kind="ExternalInput")
    d_gq = nc.dram_tensor("gq", [H], F32, kind="ExternalInput")
    d_bq = nc.dram_tensor("bq", [H], F32, kind="ExternalInput")
    d_gk = nc.dram_tensor("gk", [H], F32, kind="ExternalInput")
    d_bk = nc.dram_tensor("bk", [H], F32, kind="ExternalInput")
    d_id = nc.dram_tensor("ident", [P, P], F32R, kind="ExternalInput")
    d_out = nc.dram_tensor("out", [BPC, L, H], F32, kind="ExternalOutput")

    AF = mybir.ActivationFunctionType
    OP = mybir.AluOpType

    with tile.TileContext(nc) as tc:
        with (
            tc.tile_pool(name="const", bufs=1) as cpool,
            tc.tile_pool(name="io", bufs=2) as iopool,
            tc.tile_pool(name="acts", bufs=2) as apool,
            tc.tile_pool(name="small", bufs=3) as smpool,
            tc.tile_pool(name="psum", bufs=4, space="PSUM") as pspool,
            tc.tile_pool(name="psumt", bufs=1, space="PSUM") as tppool,
        ):
            # ---- identity + batch-0 gather DMAs first, so the PE can start
            # transposing before the 7MB of constants lands ----
            ident = cpool.tile([P, P], F32R, name="ident")
            nc.sync.dma_start(out=ident[:], in_=d_id.ap())

            emb_ap = d_emb.ap()
            idx_tiles, msk_tiles, X_tiles = {}, {}, {}

            def emit_batch_inputs(b):
                idx = iopool.tile([P, NC_CHUNKS], I32, name=f"idx{b}", tag="idx")
                nc.sync.dma_start(
                    out=idx[:], in_=d_pos.ap()[b].rearrange("(c p) -> p c", p=P)
                )
                msk = iopool.tile([P, NC_CHUNKS], F32, name=f"msk{b}", tag="msk")
                nc.sync.dma_start(
                    out=msk[:], in_=d_msk.ap()[b].rearrange("(c p) -> p c", p=P)
                )
                X = []
                last_inst = None
                for lc in range(NC_CHUNKS):
                    xg = iopool.tile([P, H], F32R, name=f"x{b}_{lc}", tag=f"x{lc}")
                    last_inst = nc.gpsimd.indirect_dma_start(
                        out=xg[:],
                        out_offset=None,
                        in_=emb_ap,
                        in_offset=bass.IndirectOffsetOnAxis(
                            ap=idx[:, lc:lc + 1], axis=0
                        ),
                    )
                    X.append(xg)
                idx_tiles[b], msk_tiles[b], X_tiles[b] = idx, msk, X
                return last_inst

            emit_batch_inputs(0)

            def load_chunks(dram, name, dt=F32):
                ts = []
                for c in range(NC_CHUNKS):
                    t = cpool.tile([P, L], dt, name=f"{name}{c}", tag=f"{name}{c}")
                    nc.sync.dma_start(out=t[:], in_=dram[c * P:(c + 1) * P, :])
                    ts.append(t)
                return ts

            PosT = load_chunks(d_post, "pt")
            WzT = load_chunks(d_wzt, "wz", F32R)
            WvT = load_chunks(d_wvt, "wv", F32R)
            WqT = load_chunks(d_wqt, "wq", F32R)
            WkT = load_chunks(d_wkt, "wk", F32R)
            M1s = load_chunks(d_m1s, "m1")
            Ds = load_chunks(d_ds, "ds")

            def load_vec(dram, name):
                t = cpool.tile([P, NC_CHUNKS], F32, name=name)
                nc.sync.dma_start(
                    out=t[:], in_=dram.ap().rearrange("(c p) -> p c", p=P)
                )
                return t

            gq = load_vec(d_gq, "gq")
            bq = load_vec(d_bq, "bq")
            gk = load_vec(d_gk, "gk")
            bk = load_vec(d_bk, "bk")

            def compute_XT(b):
                # XT = X^T + PosT : 4x [128(k), 512(l)].  lc is the outer loop
                # so the first transposes depend only on the first gather chunk.
                X = X_tiles[b]
                tps = [
                    tppool.tile([P, L], F32R, name=f"tp{b}_{kc}", tag=f"tp{kc}")
                    for kc in range(NC_CHUNKS)
                ]
                for lc in range(NC_CHUNKS):
                    for kc in range(NC_CHUNKS):
                        nc.tensor.transpose(
                            out=tps[kc][:, lc * P:(lc + 1) * P],
                            in_=X[lc][:, kc * P:(kc + 1) * P],
                            identity=ident[:],
                        )
                XT = []
                for kc in range(NC_CHUNKS):
                    xtt = apool.tile([P, L], F32R, name=f"xt{b}_{kc}", tag=f"xt{kc}")
                    nc.vector.tensor_add(out=xtt[:], in0=tps[kc][:], in1=PosT[kc][:])
                    XT.append(xtt)
                return XT

            XT_next = compute_XT(0)

            for b in range(BPC):
                msk = msk_tiles[b]
                XT = XT_next

                # ---- Z^T[h,l] = silu(sum_k Wz[h,k] XT[k,l]) ----
                ZT = []
                for hc in range(NC_CHUNKS):
                    zp = pspool.tile([P, L], F32, name=f"zp{b}_{hc}", tag="mm")
                    for kc in range(NC_CHUNKS):
                        nc.tensor.matmul(
                            out=zp[:],
                            lhsT=WzT[kc][:, hc * P:(hc + 1) * P],
                            rhs=XT[kc][:],
                            start=(kc == 0),
                            stop=(kc == NC_CHUNKS - 1),
                        )
                    zt = apool.tile([P, L], F32R, name=f"zt{b}_{hc}", tag=f"zt{hc}")
                    nc.scalar.activation(out=zt[:], in_=zp[:], func=AF.Silu)
                    ZT.append(zt)

                if b + 1 < BPC:
                    emit_batch_inputs(b + 1)

                # ---- V[l,h] = silu(sum_k XT[k,l] WvT[k,h]) ----
                V = []
                for lc in range(NC_CHUNKS):
                    vp = pspool.tile([P, L], F32, name=f"vp{b}_{lc}", tag="mm")
                    for kc in range(NC_CHUNKS):
                        nc.tensor.matmul(
                            out=vp[:],
                            lhsT=XT[kc][:, lc * P:(lc + 1) * P],
                            rhs=WvT[kc][:],
                            start=(kc == 0),
                            stop=(kc == NC_CHUNKS - 1),
                        )
                    vt = apool.tile([P, L], F32R, name=f"v{b}_{lc}", tag=f"v{lc}")
                    nc.scalar.activation(out=vt[:], in_=vp[:], func=AF.Silu)
                    V.append(vt)

                # ---- Q^T = (Wq @ Z^T) * gamma_q + beta_q ; K^T likewise ----
                QT, KT = [], []
                for (wt, gam, bet, outl, nm, eng) in (
                    (WkT, gk, bk, KT, "k", "dve"),
                    (WqT, gq, bq, QT, "q", "act"),
                ):
                    for hc in range(NC_CHUNKS):
                        qp = pspool.tile([P, L], F32, name=f"{nm}p{b}_{hc}", tag="mm")
                        for kc in range(NC_CHUNKS):
                            nc.tensor.matmul(
                                out=qp[:],
                                lhsT=wt[kc][:, hc * P:(hc + 1) * P],
                                rhs=ZT[kc][:],
                                start=(kc == 0),
                                stop=(kc == NC_CHUNKS - 1),
                            )
                        qt = apool.tile(
                            [P, L], F32R, name=f"{nm}t{b}_{hc}", tag=f"{nm}t{hc}"
                        )
                        if eng == "dve":
                            nc.vector.tensor_scalar(
                                out=qt[:],
                                in0=qp[:],
                                scalar1=gam[:, hc:hc + 1],
                                scalar2=bet[:, hc:hc + 1],
                                op0=OP.mult,
                                op1=OP.add,
                            )
                        else:
                            nc.scalar.activation(
                                out=qt[:],
                                in_=qp[:],
                                func=AF.Identity,
                                scale=gam[:, hc:hc + 1],
                                bias=bet[:, hc:hc + 1],
                            )
                        outl.append(qt)

                # next batch: transposes now, so the PE has fill work during
                # this batch's attention phase
                if b + 1 < BPC:
                    XT_next = compute_XT(b + 1)

                # ---- S_b[j,l] = M1s[j,l]*mask[j] + Ds[j,l] ----
                S = []
                for mc in range(NC_CHUNKS):
                    ms = mc * P  # columns l < 128*mc are exactly zero (causal)
                    st = apool.tile([P, L], F32, name=f"s{b}_{mc}", tag=f"s{mc}")
                    nc.vector.scalar_tensor_tensor(
                        out=st[:, ms:],
                        in0=M1s[mc][:, ms:],
                        scalar=msk[:, mc:mc + 1],
                        in1=Ds[mc][:, ms:],
                        op0=OP.mult,
                        op1=OP.add,
                    )
                    S.append(st)

                # ---- P^T[m,l] = sum_d KT[d,m] QT[d,l] ; A^T = (relu*S)^2 ----
                A = []
                for mc in range(NC_CHUNKS):
                    ms = mc * P
                    pp = pspool.tile([P, L], F32, name=f"pp{b}_{mc}", tag="mm")
                    for dc in range(NC_CHUNKS):
                        nc.tensor.matmul(
                            out=pp[:, ms:],
                            lhsT=KT[dc][:, mc * P:(mc + 1) * P],
                            rhs=QT[dc][:, ms:],
                            start=(dc == 0),
                            stop=(dc == NC_CHUNKS - 1),
                        )
                    u = smpool.tile([P, L], F32, name=f"u{b}_{mc}", tag="u")
                    nc.vector.scalar_tensor_tensor(
                        out=u[:, ms:],
                        in0=pp[:, ms:],
                        scalar=0.0,
                        in1=S[mc][:, ms:],
                        op0=OP.max,
                        op1=OP.mult,
                    )
                    at = apool.tile([P, L], F32R, name=f"a{b}_{mc}", tag=f"a{mc}")
                    nc.vector.tensor_mul(out=at[:, ms:], in0=u[:, ms:], in1=u[:, ms:])
                    A.append(at)

                # ---- OUT[l,h] = sum_m A[m,l] V[m,h] ----
                for lc in range(NC_CHUNKS):
                    op_ = pspool.tile([P, L], F32, name=f"op{b}_{lc}", tag="mm")
                    for mc in range(lc + 1):  # A[mc] chunk is zero for mc > lc
                        nc.tensor.matmul(
                            out=op_[:],
                            lhsT=A[mc][:, lc * P:(lc + 1) * P],
                            rhs=V[mc][:],
                            start=(mc == 0),
                            stop=(mc == lc),
                        )
                    ot = smpool.tile([P, L], F32, name=f"o{b}_{lc}", tag="o", bufs=8)
                    if b == BPC - 1 and lc % 2 == 1:
                        nc.vector.tensor_copy(out=ot[:], in_=op_[:])
                    else:
                        nc.scalar.copy(out=ot[:], in_=op_[:])
                    nc.sync.dma_start(
                        out=d_out.ap()[b, lc * P:(lc + 1) * P, :], in_=ot[:]
                    )

    nc.compile()
    return nc


def _host_prep(positives, mask, item_emb, pos_emb, Wz, Wv, Wq, Wk,
               gamma_q, beta_q, gamma_k, beta_k, sparse_w, gumbel):
    """Small O(L^2) constant prep + per-core input shards."""
    f32 = np.float32
    positives = np.ascontiguousarray(np.asarray(positives).astype(np.int32))
    maskf = np.ascontiguousarray(np.asarray(mask).astype(f32))
    item_emb = np.ascontiguousarray(np.asarray(item_emb, f32))
    pos_emb = np.asarray(pos_emb, f32)
    sw = np.asarray(sparse_w, f32)
    gum = np.asarray(gumbel, f32)

    smask = (1.0 / (1.0 + np.exp(-((np.log(sw / (1.0 - sw)) + gum) / f32(TEMP)))))
    smask = smask.astype(f32)
    scl = f32(1.0 / np.sqrt(L * H))
    j = np.arange(L)
    strict_lower_T = (j[:, None] < j[None, :])  # [j, l] : j < l
    M1s = np.ascontiguousarray((smask.T * strict_lower_T * scl).astype(f32))
    Ds = np.ascontiguousarray((np.diag(np.diag(smask)) * scl).astype(f32))

    shared = {
        "ident": np.eye(P, dtype=f32),
        "item_emb": item_emb,
        "PosT": np.ascontiguousarray(pos_emb.T.astype(f32)),
        "WzT": np.ascontiguousarray(np.asarray(Wz, f32).T),
        "WvT": np.ascontiguousarray(np.asarray(Wv, f32).T),
        "WqT": np.ascontiguousarray(np.asarray(Wq, f32).T),
        "WkT": np.ascontiguousarray(np.asarray(Wk, f32).T),
        "M1s": M1s,
        "Ds": Ds,
        "gq": np.ascontiguousarray(np.asarray(gamma_q, f32)),
        "bq": np.ascontiguousarray(np.asarray(beta_q, f32)),
        "gk": np.ascontiguousarray(np.asarray(gamma_k, f32)),
        "bk": np.ascontiguousarray(np.asarray(beta_k, f32)),
    }
    in_maps = []
    for c in range(N_CORES):
        sl = slice(c * BPC, (c + 1) * BPC)
        m = dict(shared)
        m["positives"] = positives[sl]
        m["maskf"] = maskf[sl]
        in_maps.append(m)
    return in_maps


def get_module():
    global _COMPILED
    if _COMPILED is None:
        _COMPILED = _build_module()
    return _COMPILED


def kernel(**inputs) -> np.ndarray:
    nc = get_module()
    in_maps = _host_prep(**inputs)
    res = run_bass_kernel_spmd(nc, in_maps, core_ids=list(range(N_CORES)))
    out = np.concatenate([r["out"] for r in res.results], axis=0)
    return out.astype(np.float32)


if __name__ == "__main__":
    rng = np.random.default_rng(0)
    demo = {
        "positives": rng.integers(0, ITEM, (B, L)).astype(np.int32),
        "mask": rng.integers(0, 2, (B, L)).astype(np.int32),
        "item_emb": rng.normal(size=(ITEM, H)).astype(np.float32) * 0.02,
        "pos_emb": rng.normal(size=(L, H)).astype(np.float32) * 0.02,
        "Wz": rng.normal(size=(L, L)).astype(np.float32),
        "Wv": rng.normal(size=(L, L)).astype(np.float32),
        "Wq": rng.normal(size=(L, L)).astype(np.float32),
        "Wk": rng.normal(size=(L, L)).astype(np.float32),
        "gamma_q": rng.normal(size=(L,)).astype(np.float32) * 0.02,
        "beta_q": np.zeros((L,), np.float32),
        "gamma_k": rng.normal(size=(L,)).astype(np.float32) * 0.02,
        "beta_k": np.zeros((L,), np.float32),
        "sparse_w": rng.uniform(0.2, 0.8, (L, H)).astype(np.float32),
        "gumbel": rng.normal(size=(L, H)).astype(np.float32),
    }
    out = kernel(**demo)
    print("out", out.shape, out.dtype, np.abs(out).max())

